# revision 11
# baseline (speedup 1.0000x reference)
"""Trainium2 Bass kernel for nn_Melody_RNN (B=64, S=512, A=20, V=130, E=H=64, L=2).

Structure exploited (all implied by the reference's exact semantics):
  * Only embedding rows for inputs[:,0] / inputs[:,1] are used; the LSTM runs
    exactly 2 timesteps (zero initial state, so the forget gate is dead).
  * The torch cat+view memory reinterpretations make every output row depend
    only on s%64 (plus batch-0 specials for s<84), so the unique content is
    og[84,130] (generic) + ob[84,130] (batch-0 head, core 0 only).
  * The attention-mask bug makes softmax exactly uniform.

Kernel v9 notes (vs v8, 28.5us measured):
  * Middle section fully matmul-ized: tables are computed TRANSPOSED
    (tabT[j=128,h]) so the sliding-window-20 sum, the Wc column selection
    and the Whb/Wcb bias terms all become one 3-matmul psum accumulation
    against host-packed count/selection/bias matrices [128,168]
    (cols 0:84 generic, 84:168 batch-0 variant).  Replaces the shift-add
    window tree + 12 bias-add copies of v8.
  * Batch-0 blend for the attention half is gone: core 0 simply gets
    different packed constants (CZh/CSelZ) than cores 1-7 (CGh/CSelG).
    Only the outputs-half keeps a 2-op early blend via mvec.
  * og/ob decoded in parallel psum banks into one ogob[84,260] tile.
    Slot rows 0:84 (chunks 0:21) map 1:1 to og rows, so the head DMAs
    read ogob directly (260B descriptors, half-rate but tiny);
    only chunks 21:128 need the partition-regrouping selection matmuls.
  * Output: big DMA covers chunks 21:128 of ALL 8 slots (slot-0 tail is
    generic), column-split into two pipelined DMAs; head7 writes generic
    rows 0:84 into slots 1-7; head0 writes ob rows into slot 0.
  * Single ACT table load: one sigmoid dummy first on the ACT queue makes
    the compiler pick act-func-set 2 (sigmoid_and_others) which also
    contains tanh/identity/copy.

SPMD: 8 cores, identical program; per-core input differs in the window/
selection constant columns 84:168 of packc and the mvec blend columns.
"""

import sys
import numpy as np

if "/root/.axon_site/_ro/trn_rl_repo" not in sys.path:
    sys.path.insert(0, "/root/.axon_site/_ro/trn_rl_repo")

B, S, A = 64, 512, 20
V, E, H = 130, 64, 64
NCORES = 8
BPC = B // NCORES  # batches (slots) per core

# packa column layout (bf16, 64 partitions)
_XS = 0            # [64, 128] = [x0T | x1T]
_WIH0 = 128        # [64, 192] gates (i,g,o)
_WIH1 = 320        # [64, 192]
_WHW = 512         # [64, 128] = [Whw[:, :64].T | Whw[:, 64:].T] pre-scaled 1/A
_WCW = 640         # [64, 128]
_PAW = 768

# packm (f32, 128 partitions): LSTM gate biases + blend scalars
# cols: 0: l0 [i|g], 1: l0 o, 2: l1 [i|g], 3: l1 o, 4: mvec, 5: 1-mvec
_PMW = 8

# packc column layout (bf16, 128 partitions)
_CG = 0            # [128, 168] window counts: generic | per-core (CZh on core 0)
_CS = 168          # [128, 168] Wc selection: generic | per-core
_BIAS = 336        # [2, 64] rows: Whb/A, Wcb
_CNT = 400         # [2, 168] rows: min(t,20), ones
_ONES = 568        # [1, 84] ones (decb row source for outG/outB row 64)
_UB = 652          # 4 x [84, 107] selection for chunks 21:128, sub-row j
_DECA = 1080       # [64, 130] decw[:, 0:64].T
_DECB = 1210       # [65, 130] decw[:, 64:128].T + decb row
_PCW = 1340

SLOT = S * V       # elements per output slot (66560)
CW = 4 * V         # chunk width in elements (520)
NB = 107           # tail chunks (21..127)

_NC_CACHE = {}


def _build_nc():
    import concourse.bass as bass
    import concourse.bacc as bacc
    import concourse.mybir as mybir
    from concourse.tile import TileContext

    f32 = mybir.dt.float32
    bf16 = mybir.dt.bfloat16
    AF = mybir.ActivationFunctionType

    nc = bacc.Bacc("TRN2", target_bir_lowering=False, debug=False)

    d_pa = nc.dram_tensor("packa", [64, _PAW], bf16, kind="ExternalInput")
    d_pm = nc.dram_tensor("packm", [128, _PMW], f32, kind="ExternalInput")
    d_pc = nc.dram_tensor("packc", [128, _PCW], bf16, kind="ExternalInput")
    d_out = nc.dram_tensor("out", [BPC * S, V], bf16, kind="ExternalOutput")

    with TileContext(nc) as tc:
        with (
            tc.tile_pool(name="sbuf", bufs=1) as pool,
            tc.tile_pool(name="psum", bufs=1, space="PSUM") as pp,
        ):
            # ---- input loads: sync queue gets the LSTM-critical columns ----
            pa = pool.tile([64, _PAW], bf16)
            pm = pool.tile([128, _PMW], f32)
            pc = pool.tile([128, _PCW], bf16)
            nc.sync.dma_start(out=pa[:, 0:_WIH1], in_=d_pa[:, 0:_WIH1])
            nc.scalar.dma_start(out=pm[:], in_=d_pm[:])
            nc.sync.dma_start(out=pa[:, _WIH1:_PAW], in_=d_pa[:, _WIH1:_PAW])
            nc.scalar.dma_start(out=pc[:], in_=d_pc[:])

            xs = pa[:, _XS:_XS + 128]
            wih0 = pa[:, _WIH0:_WIH0 + 192]
            wih1 = pa[:, _WIH1:_WIH1 + 192]
            whw = pa[:, _WHW:_WHW + 128]
            wcw = pa[:, _WCW:_WCW + 128]
            biasp = pm[:, 0:8]

            # ---- LSTM: both steps batched; hcat/ccat hold 2h / 2c
            #      (tanh-only formulation: sigma(x) = (tanh(x/2)+1)/2, the
            #      x2 folds into consumers on the host).  Tanh-only keeps the
            #      act-table pass on ONE hoisted set-0 load.
            #      cols [l0s0(b) | l0s1(b) | l1s0(b) | l1s1(b)]
            hcat = pool.tile([H, 256], bf16)
            ccat = pool.tile([H, 256], bf16)
            MUL = mybir.AluOpType.mult
            ADD = mybir.AluOpType.add

            def lstm_layer(rhsT, wp, bc, dst_off, tag, insc):
                # insc: 1.0 when rhsT holds true inputs, 0.5 when it holds 2h
                ps0 = pp.tile([128, 128], f32, tag="gates")   # [i|g]
                ps1 = pp.tile([64, 128], f32, tag="gateso")   # [o]
                nc.tensor.matmul(ps0[:], wp[:, 0:128], rhsT, start=True, stop=True)
                nc.tensor.matmul(ps1[:], wp[:, 128:192], rhsT, start=True, stop=True)
                tanh_i = pool.tile([H, 128], bf16, tag=f"ti{tag}")
                tanh_g = pool.tile([H, 128], bf16, tag=f"tg{tag}")
                tanh_o = pool.tile([H, 128], bf16, tag=f"to{tag}")
                tanh_c = pool.tile([H, 128], bf16, tag=f"tc{tag}")
                cc = ccat[:, dst_off:dst_off + 128]  # holds 2c
                hh = hcat[:, dst_off:dst_off + 128]  # holds 2h
                # biases packed as b_i/2, b_g, b_o/2
                nc.scalar.activation(tanh_g[:], ps0[64:128, :], AF.Tanh,
                                     bias=biasp[64:128, bc:bc + 1], scale=insc)
                nc.scalar.activation(tanh_i[:], ps0[0:64, :], AF.Tanh,
                                     bias=biasp[0:64, bc:bc + 1], scale=0.5 * insc)
                # 2c = (tanh_i + 1) * tanh_g
                nc.vector.scalar_tensor_tensor(cc, tanh_i[:], 1.0, tanh_g[:],
                                               ADD, MUL)
                nc.scalar.activation(tanh_o[:], ps1[0:64, :], AF.Tanh,
                                     bias=biasp[0:64, bc + 1:bc + 2],
                                     scale=0.5 * insc)
                nc.scalar.activation(tanh_c[:], cc, AF.Tanh, scale=0.5)
                # 2h = (tanh_o + 1) * tanh_c
                nc.vector.scalar_tensor_tensor(hh, tanh_o[:], 1.0, tanh_c[:],
                                               ADD, MUL)

            lstm_layer(xs, wih0, 0, 0, "l0", 1.0)
            lstm_layer(hcat[:, 0:128], wih1, 2, 128, "l1", 0.5)
            out0T = hcat[:, 128:192]   # l1 s0
            out1T = hcat[:, 192:256]   # l1 s1

            # ---- transposed tables tabT[j=128, 0:64]=h, [.., 64:128]=c ----
            # layer-0 half emitted after the l1 gates matmuls so the PE runs
            # it during the l1 activation chain.
            tabP = pp.tile([128, 128], f32, tag="tab")

            def tab_mms(L):
                lo = 128 * L
                for src, w, coff in ((hcat, whw, 0), (ccat, wcw, 64)):
                    nc.tensor.matmul(tabP[64 * L:64 * L + 64, coff:coff + 64],
                                     src[:, lo + 0:lo + 128:2], w[:, 0:64],
                                     start=True, stop=False)
                    nc.tensor.matmul(tabP[64 * L:64 * L + 64, coff:coff + 64],
                                     src[:, lo + 1:lo + 128:2], w[:, 64:128],
                                     start=False, stop=True)

            tab_mms(0)

            # ---- outputs-half tiles (early; gpsimd/vector are idle) ----
            outG = pool.tile([65, 84], bf16)
            outB = pool.tile([65, 84], bf16)
            tmpB = pool.tile([64, 64], bf16)
            nc.gpsimd.tensor_copy(outG[0:64, 0:64], out1T)
            nc.gpsimd.tensor_copy(outG[0:64, 64:84], out1T[:, 0:20])
            nc.gpsimd.tensor_copy(outB[0:64, 64:84], out1T[:, 0:20])
            nc.scalar.copy(outG[64:65, :], pc[0:1, _ONES:_ONES + 84])
            nc.scalar.copy(outB[64:65, :], pc[0:1, _ONES:_ONES + 84])
            nc.vector.tensor_scalar_mul(tmpB[:], out0T, pm[0:64, 4:5])
            nc.vector.scalar_tensor_tensor(outB[0:64, 0:64], out1T,
                                           pm[0:64, 5:6], tmpB[:],
                                           MUL, ADD)

            tab_mms(1)

            tabs = pool.tile([128, 128], bf16)
            nc.vector.tensor_copy(tabs[:, 0:64], tabP[:, 0:64])
            nc.scalar.copy(tabs[:, 64:128], tabP[:, 64:128])

            # ---- attn halves via 3 accumulating matmuls [64, 168] ----
            attnP = pp.tile([64, 168], f32, tag="attn")
            nc.tensor.matmul(attnP[:], tabs[:, 0:64], pc[:, _CG:_CG + 168],
                             start=True, stop=False)
            nc.tensor.matmul(attnP[:], tabs[:, 64:128], pc[:, _CS:_CS + 168],
                             start=False, stop=False)
            nc.tensor.matmul(attnP[:], pc[0:2, _BIAS:_BIAS + 64],
                             pc[0:2, _CNT:_CNT + 168], start=False, stop=True)
            attns = pool.tile([64, 168], bf16)
            nc.vector.tensor_copy(attns[:, 0:84], attnP[:, 0:84])
            nc.scalar.copy(attns[:, 84:168], attnP[:, 84:168])

            # ---- decode og/ob in parallel psum banks ----
            deca = pc[0:64, _DECA:_DECA + V]
            decb2 = pc[0:65, _DECB:_DECB + V]
            ogP = pp.tile([84, V], f32, tag="og")
            obP = pp.tile([84, V], f32, tag="ob")
            nc.tensor.matmul(ogP[:], outG[:], decb2, start=True, stop=False)
            nc.tensor.matmul(ogP[:], attns[0:64, 0:84], deca, start=False, stop=True)
            nc.tensor.matmul(obP[:], outB[:], decb2, start=True, stop=False)
            nc.tensor.matmul(obP[:], attns[0:64, 84:168], deca, start=False, stop=True)
            ogob = pool.tile([84, 260], bf16)
            nc.vector.tensor_copy(ogob[:, 0:V], ogP[:])
            nc.scalar.copy(ogob[:, V:260], obP[:])
            og_b = ogob[:, 0:V]
            ogobT = ogob[:].tensor

            # ---- chunks 21:128 via selection matmuls (partition regroup) --
            pb01 = pp.tile([NB, 260], f32, tag="pb01")
            pb23 = pp.tile([NB, 260], f32, tag="pb23")
            for j, (dst, off) in enumerate(
                    [(pb01, 0), (pb01, V), (pb23, 0), (pb23, V)]):
                nc.tensor.matmul(dst[:, off:off + V],
                                 pc[0:84, _UB + 107 * j:_UB + 107 * j + 107],
                                 og_b, start=True, stop=True)
            ogB = pool.tile([NB, CW], bf16)
            nc.vector.tensor_copy(ogB[:, 0:260], pb01[:])
            nc.scalar.copy(ogB[:, 260:CW], pb23[:])
            ogBT = ogB[:].tensor

            # ---- output DMAs ----
            # tail (chunks 21:128) of ALL 8 slots, column-split so the first
            # half ships while the second is still casting.
            nc.sync.dma_start(
                out=bass.AP(d_out, 84 * V, [[CW, NB], [SLOT, 8], [1, 260]]),
                in_=bass.AP(ogBT, 0, [[CW, NB], [0, 8], [1, 260]]))
            nc.sync.dma_start(
                out=bass.AP(d_out, 84 * V + 260, [[CW, NB], [SLOT, 8], [1, 260]]),
                in_=bass.AP(ogBT, 260, [[CW, NB], [0, 8], [1, 260]]))
            # heads: rows 0:84 map 1:1 to og/ob rows -> DMA straight from ogob
            nc.scalar.dma_start(
                out=bass.AP(d_out, SLOT, [[V, 84], [SLOT, 7], [1, V]]),
                in_=bass.AP(ogobT, 0, [[260, 84], [0, 7], [1, V]]))
            nc.scalar.dma_start(
                out=bass.AP(d_out, 0, [[V, 84], [1, V]]),
                in_=bass.AP(ogobT, V, [[260, 84], [1, V]]))

    nc.compile()
    return nc


def _get_nc():
    if "nc" not in _NC_CACHE:
        _NC_CACHE["nc"] = _build_nc()
    return _NC_CACHE["nc"]


def _host_reference_fallback(inputs):
    """Pure-numpy replica of the reference for steps != 512 (never hit with the
    canonical setup_inputs, which fixes lengths = 512)."""
    emb = inputs["emb"]; L = 2
    Ls = np.asarray(inputs["lengths"]); steps = int(Ls.max()); batch = inputs["inputs"].shape[0]
    layers = [(inputs["Wih0"], inputs["bih0"], inputs["bhh0"]),
              (inputs["Wih1"], inputs["bih1"], inputs["bhh1"])]
    sig = lambda z: 1.0 / (1.0 + np.exp(-z))

    def step(x):
        hs, cs = [], []
        inp = x
        for (Wih, bih, bhh) in layers:
            g = inp @ Wih.T + bih + bhh
            i, f, gg, o = np.split(g, 4, axis=-1)
            c = sig(i) * np.tanh(gg)
            h = sig(o) * np.tanh(c)
            hs.append(h); cs.append(c); inp = h
        return inp.astype(np.float32), np.stack(hs).astype(np.float32), np.stack(cs).astype(np.float32)

    x0 = emb[inputs["inputs"][:, 0]]
    x1 = emb[inputs["inputs"][:, 1]]
    out0, h0, c0 = step(x0)
    out1, h1, c1 = step(x1)
    outputs = np.concatenate(
        [out0[None], np.broadcast_to(out1[None], (steps - 1, batch, H))], 0
    ).reshape(batch, steps, H)
    h_steps = np.concatenate(
        [h0, np.broadcast_to(h1[None], (steps - 1, L, batch, H)).reshape((steps - 1) * L, batch, H)], 0
    ).reshape(batch, steps, L * H)
    c_steps = np.concatenate(
        [c0, np.broadcast_to(c1[None], (steps - 1, L, batch, H)).reshape((steps - 1) * L, batch, H)], 0
    ).reshape(batch, steps, L * H)
    Wh = h_steps @ inputs["Whw"].T + inputs["Whb"]
    Wc = c_steps @ inputs["Wcw"].T + inputs["Wcb"]
    idx = np.arange(steps)[:, None] + np.arange(A)[None, :] - A
    valid = idx >= 0
    win = np.where(valid[None, :, :, None], Wh[:, np.clip(idx, 0, None)], 0.0)
    att = win + Wc[:, :, None, :]
    attn = att.mean(axis=2)
    concat_h = np.concatenate([attn, outputs], axis=2)
    outs = concat_h @ inputs["decw"].T + inputs["decb"]
    bi, ti = np.nonzero(np.arange(steps)[None, :] < (Ls[:, None] - 1))
    return outs[bi, ti].reshape(-1, V).astype(np.float32)


def _g_generic(t):
    u = t % 64
    return 32 + u if u < 32 else 96 + (u - 32)


def _g_b0(t):
    if t >= 64:
        return 32 + (t - 64)
    return t if t < 32 else 64 + (t - 32)


def _window_counts(gfun):
    C = np.zeros((128, 84), np.float32)
    for t in range(84):
        for s in range(max(0, t - 20), t):
            C[gfun(s), t] += 1.0
    return C


def _selection(gfun):
    C = np.zeros((128, 84), np.float32)
    for t in range(84):
        C[gfun(t), t] = 1.0
    return C


def _pack_inputs(inputs):
    import ml_dtypes
    f32 = np.float32
    bf = ml_dtypes.bfloat16
    emb = inputs["emb"].astype(f32)
    idx0 = np.asarray(inputs["inputs"][:, 0]).astype(np.int64)
    idx1 = np.asarray(inputs["inputs"][:, 1]).astype(np.int64)

    def gates_pack(Wih):
        W = np.asarray(Wih, dtype=f32)
        return np.concatenate([W[0:H], W[2 * H:3 * H], W[3 * H:4 * H]], axis=0).T

    pa = np.zeros((64, _PAW), f32)
    pa[:, _XS:_XS + 64] = emb[idx0].T
    pa[:, _XS + 64:_XS + 128] = emb[idx1].T
    pa[:, _WIH0:_WIH0 + 192] = gates_pack(inputs["Wih0"])
    pa[:, _WIH1:_WIH1 + 192] = gates_pack(inputs["Wih1"])
    # hcat/ccat hold 2h/2c -> fold the 0.5 into every consumer here
    Whw = np.asarray(inputs["Whw"], f32)
    Wcw = np.asarray(inputs["Wcw"], f32)
    pa[:, _WHW:_WHW + 64] = Whw[:, 0:H].T / (2 * A)
    pa[:, _WHW + 64:_WHW + 128] = Whw[:, H:2 * H].T / (2 * A)
    pa[:, _WCW:_WCW + 64] = Wcw[:, 0:H].T / 2
    pa[:, _WCW + 64:_WCW + 128] = Wcw[:, H:2 * H].T / 2
    pa = pa.astype(bf)

    # gate biases packed as b_i/2, b_g, b_o/2 (tanh-only formulation)
    pmb = np.zeros((128, _PMW), f32)
    b0 = np.asarray(inputs["bih0"], f32) + np.asarray(inputs["bhh0"], f32)
    b1 = np.asarray(inputs["bih1"], f32) + np.asarray(inputs["bhh1"], f32)
    pmb[0:64, 0] = b0[0:H] / 2
    pmb[64:128, 0] = b0[2 * H:3 * H]
    pmb[0:64, 1] = b0[3 * H:4 * H] / 2
    pmb[0:64, 2] = b1[0:H] / 2
    pmb[64:128, 2] = b1[2 * H:3 * H]
    pmb[0:64, 3] = b1[3 * H:4 * H] / 2
    pmb[0:64, 5] = 1.0  # 1-mvec (generic cores); core 0 overridden below

    decw = np.asarray(inputs["decw"], f32)
    CGh = _window_counts(_g_generic)
    CZh = _window_counts(_g_b0)
    CSelG = _selection(_g_generic)
    CSelZ = _selection(_g_b0)
    cnt = np.minimum(np.arange(84), 20).astype(f32)

    def packc_for(core):
        p = np.zeros((128, _PCW), f32)
        p[:, _CG:_CG + 84] = CGh
        p[:, _CG + 84:_CG + 168] = CZh if core == 0 else CGh
        p[:, _CS:_CS + 84] = CSelG
        p[:, _CS + 84:_CS + 168] = CSelZ if core == 0 else CSelG
        p[0, _BIAS:_BIAS + 64] = np.asarray(inputs["Whb"], f32) / A
        p[1, _BIAS:_BIAS + 64] = np.asarray(inputs["Wcb"], f32)
        p[0, _CNT:_CNT + 84] = cnt
        p[0, _CNT + 84:_CNT + 168] = cnt
        p[1, _CNT:_CNT + 168] = 1.0
        p[0, _ONES:_ONES + 84] = 1.0
        for j in range(4):
            for q in range(NB):
                p[20 + (4 * q) % 64 + j, _UB + 107 * j + q] = 1.0
        p[0:64, _DECA:_DECA + V] = decw[:, 0:H].T
        p[0:64, _DECB:_DECB + V] = decw[:, H:2 * H].T / 2  # outG/outB hold 2h
        p[64, _DECB:_DECB + V] = np.asarray(inputs["decb"], f32)
        return p.astype(bf)

    pc0 = packc_for(0)
    pcg = packc_for(1)
    pm0 = pmb.copy()
    pm0[0:64, 4] = 1.0
    pm0[0:64, 5] = 0.0
    in_maps = []
    for core in range(NCORES):
        in_maps.append({"packa": pa,
                        "packm": pm0 if core == 0 else pmb,
                        "packc": pc0 if core == 0 else pcg})
    return in_maps


def kernel(**inputs):
    inputs = {k: np.asarray(v) for k, v in inputs.items()}
    Ls = np.asarray(inputs["lengths"]).astype(np.int64)
    steps = int(Ls.max())
    if steps != S or inputs["inputs"].shape != (B, S):
        return _host_reference_fallback(inputs)

    from concourse.bass_utils import run_bass_kernel_spmd

    in_maps = _pack_inputs(inputs)
    nc = _get_nc()
    res = run_bass_kernel_spmd(nc, in_maps, core_ids=list(range(NCORES)))
    outs = np.concatenate(
        [r["out"].astype(np.float32).reshape(BPC, S, V) for r in res.results],
        axis=0)  # [64,512,130]

    bi, ti = np.nonzero(np.arange(steps)[None, :] < (Ls[:, None] - 1))
    return np.ascontiguousarray(outs[bi, ti].reshape(-1, V))


# revision 13
# speedup vs baseline: 1.1364x; 1.1364x over previous
"""Trainium2 Bass kernel for nn_Melody_RNN (B=64, S=512, A=20, V=130, E=H=64, L=2).

Structure exploited (all implied by the reference's exact semantics):
  * Only embedding rows for inputs[:,0] / inputs[:,1] are used; the LSTM runs
    exactly 2 timesteps (zero initial state, so the forget gate is dead).
  * The torch cat+view memory reinterpretations make every output row depend
    only on s%64 (plus batch-0 specials for s<84), so the unique content is
    og[84,130] (generic) + ob[84,130] (batch-0 head, core 0 only).
  * The attention-mask bug makes softmax exactly uniform.

Kernel v9 notes (vs v8, 28.5us measured):
  * Middle section fully matmul-ized: tables are computed TRANSPOSED
    (tabT[j=128,h]) so the sliding-window-20 sum, the Wc column selection
    and the Whb/Wcb bias terms all become one 3-matmul psum accumulation
    against host-packed count/selection/bias matrices [128,168]
    (cols 0:84 generic, 84:168 batch-0 variant).  Replaces the shift-add
    window tree + 12 bias-add copies of v8.
  * Batch-0 blend for the attention half is gone: core 0 simply gets
    different packed constants (CZh/CSelZ) than cores 1-7 (CGh/CSelG).
    Only the outputs-half keeps a 2-op early blend via mvec.
  * og/ob decoded in parallel psum banks into one ogob[84,260] tile.
    Slot rows 0:84 (chunks 0:21) map 1:1 to og rows, so the head DMAs
    read ogob directly (260B descriptors, half-rate but tiny);
    only chunks 21:128 need the partition-regrouping selection matmuls.
  * Output: big DMA covers chunks 21:128 of ALL 8 slots (slot-0 tail is
    generic), column-split into two pipelined DMAs; head7 writes generic
    rows 0:84 into slots 1-7; head0 writes ob rows into slot 0.
  * Single ACT table load: one sigmoid dummy first on the ACT queue makes
    the compiler pick act-func-set 2 (sigmoid_and_others) which also
    contains tanh/identity/copy.

SPMD: 8 cores, identical program; per-core input differs in the window/
selection constant columns 84:168 of packc and the mvec blend columns.
"""

import sys
import numpy as np

if "/root/.axon_site/_ro/trn_rl_repo" not in sys.path:
    sys.path.insert(0, "/root/.axon_site/_ro/trn_rl_repo")

B, S, A = 64, 512, 20
V, E, H = 130, 64, 64
NCORES = 8
BPC = B // NCORES  # batches (slots) per core

# packa column layout (bf16, 64 partitions)
_XS = 0            # [64, 128] = [x0T | x1T]
_WIH0 = 128        # [64, 192] gates (i,g,o)
_WIH1 = 320        # [64, 192]
_WHW = 512         # [64, 128] = [Whw[:, :64].T | Whw[:, 64:].T] pre-scaled 1/A
_WCW = 640         # [64, 128]
_PAW = 768

# packm (f32, 128 partitions): LSTM gate biases + blend scalars
# cols: 0: l0 [i|g], 1: l0 o, 2: l1 [i|g], 3: l1 o, 4: mvec, 5: 1-mvec
_PMW = 8

# packc column layout (bf16, 128 partitions)
_CG = 0            # [128, 168] window counts: generic | per-core (CZh on core 0)
_CS = 168          # [128, 168] Wc selection: generic | per-core
_BIAS = 336        # [2, 64] rows: Whb/A, Wcb
_CNT = 400         # [2, 168] rows: min(t,20), ones
_ONES = 568        # [1, 84] ones (decb row source for outG/outB row 64)
_UB = 652          # 4 x [84, 107] selection for chunks 21:128, sub-row j
_DECA = 1080       # [64, 130] decw[:, 0:64].T
_DECB = 1210       # [65, 130] decw[:, 64:128].T + decb row
_PCW = 1340

SLOT = S * V       # elements per output slot (66560)
CW = 4 * V         # chunk width in elements (520)
NB = 107           # tail chunks (21..127)

_NC_CACHE = {}


def _build_nc():
    import concourse.bass as bass
    import concourse.bacc as bacc
    import concourse.mybir as mybir
    from concourse.tile import TileContext

    f32 = mybir.dt.float32
    bf16 = mybir.dt.bfloat16
    AF = mybir.ActivationFunctionType

    nc = bacc.Bacc("TRN2", target_bir_lowering=False, debug=False)

    d_pa = nc.dram_tensor("packa", [64, _PAW], bf16, kind="ExternalInput")
    d_pm = nc.dram_tensor("packm", [128, _PMW], f32, kind="ExternalInput")
    d_pc = nc.dram_tensor("packc", [128, _PCW], bf16, kind="ExternalInput")
    d_out = nc.dram_tensor("out", [BPC * S, V], bf16, kind="ExternalOutput")

    with TileContext(nc) as tc:
        with (
            tc.tile_pool(name="sbuf", bufs=1) as pool,
            tc.tile_pool(name="psum", bufs=1, space="PSUM") as pp,
        ):
            # ---- input loads: sync queue gets the LSTM-critical columns ----
            pa = pool.tile([64, _PAW], bf16)
            pm = pool.tile([128, _PMW], f32)
            pc = pool.tile([128, _PCW], bf16)
            nc.sync.dma_start(out=pa[:, 0:_WIH1], in_=d_pa[:, 0:_WIH1])
            nc.scalar.dma_start(out=pm[:], in_=d_pm[:])
            nc.sync.dma_start(out=pa[:, _WIH1:_PAW], in_=d_pa[:, _WIH1:_PAW])
            nc.scalar.dma_start(out=pc[:], in_=d_pc[:])

            xs = pa[:, _XS:_XS + 128]
            wih0 = pa[:, _WIH0:_WIH0 + 192]
            wih1 = pa[:, _WIH1:_WIH1 + 192]
            whw = pa[:, _WHW:_WHW + 128]
            wcw = pa[:, _WCW:_WCW + 128]
            biasp = pm[:, 0:8]

            # ---- LSTM: both steps batched; hcat/ccat hold 2h / 2c
            #      (tanh-only formulation: sigma(x) = (tanh(x/2)+1)/2, the
            #      x2 folds into consumers on the host).  Tanh-only keeps the
            #      act-table pass on ONE hoisted set-0 load.
            #      cols [l0s0(b) | l0s1(b) | l1s0(b) | l1s1(b)]
            hcat = pool.tile([H, 256], bf16)
            ccat = pool.tile([H, 256], bf16)
            MUL = mybir.AluOpType.mult
            ADD = mybir.AluOpType.add

            def lstm_layer(rhsT, wp, bc, dst_off, tag, insc):
                # insc: 1.0 when rhsT holds true inputs, 0.5 when it holds 2h
                ps0 = pp.tile([128, 128], f32, tag="gates")   # [i|g]
                ps1 = pp.tile([64, 128], f32, tag="gateso")   # [o]
                nc.tensor.matmul(ps0[:], wp[:, 0:128], rhsT, start=True, stop=True)
                nc.tensor.matmul(ps1[:], wp[:, 128:192], rhsT, start=True, stop=True)
                tanh_i = pool.tile([H, 128], bf16, tag=f"ti{tag}")
                tanh_g = pool.tile([H, 128], bf16, tag=f"tg{tag}")
                tanh_o = pool.tile([H, 128], bf16, tag=f"to{tag}")
                tanh_c = pool.tile([H, 128], bf16, tag=f"tc{tag}")
                cc = ccat[:, dst_off:dst_off + 128]  # holds 2c
                hh = hcat[:, dst_off:dst_off + 128]  # holds 2h
                # biases packed as b_i/2, b_g, b_o/2
                nc.scalar.activation(tanh_g[:], ps0[64:128, :], AF.Tanh,
                                     bias=biasp[64:128, bc:bc + 1], scale=insc)
                nc.scalar.activation(tanh_i[:], ps0[0:64, :], AF.Tanh,
                                     bias=biasp[0:64, bc:bc + 1], scale=0.5 * insc)
                # 2c = (tanh_i + 1) * tanh_g
                nc.vector.scalar_tensor_tensor(cc, tanh_i[:], 1.0, tanh_g[:],
                                               ADD, MUL)
                nc.scalar.activation(tanh_o[:], ps1[0:64, :], AF.Tanh,
                                     bias=biasp[0:64, bc + 1:bc + 2],
                                     scale=0.5 * insc)
                nc.scalar.activation(tanh_c[:], cc, AF.Tanh, scale=0.5)
                # 2h = (tanh_o + 1) * tanh_c
                nc.vector.scalar_tensor_tensor(hh, tanh_o[:], 1.0, tanh_c[:],
                                               ADD, MUL)

            lstm_layer(xs, wih0, 0, 0, "l0", 1.0)
            lstm_layer(hcat[:, 0:128], wih1, 2, 128, "l1", 0.5)
            out0T = hcat[:, 128:192]   # l1 s0
            out1T = hcat[:, 192:256]   # l1 s1

            # ---- transposed tables tabT[j=128, 0:64]=h, [.., 64:128]=c ----
            # layer-0 half emitted after the l1 gates matmuls so the PE runs
            # it during the l1 activation chain.
            tabP = pp.tile([128, 128], f32, tag="tab")

            def tab_mms(L):
                lo = 128 * L
                for src, w, coff in ((hcat, whw, 0), (ccat, wcw, 64)):
                    nc.tensor.matmul(tabP[64 * L:64 * L + 64, coff:coff + 64],
                                     src[:, lo + 0:lo + 128:2], w[:, 0:64],
                                     start=True, stop=False)
                    nc.tensor.matmul(tabP[64 * L:64 * L + 64, coff:coff + 64],
                                     src[:, lo + 1:lo + 128:2], w[:, 64:128],
                                     start=False, stop=True)

            tab_mms(0)

            # ---- outputs-half tiles (early; gpsimd/vector are idle) ----
            outG = pool.tile([65, 84], bf16)
            outB = pool.tile([65, 84], bf16)
            tmpB = pool.tile([64, 64], bf16)
            nc.gpsimd.tensor_copy(outG[0:64, 0:64], out1T)
            nc.gpsimd.tensor_copy(outG[0:64, 64:84], out1T[:, 0:20])
            nc.gpsimd.tensor_copy(outB[0:64, 64:84], out1T[:, 0:20])
            nc.scalar.copy(outG[64:65, :], pc[0:1, _ONES:_ONES + 84])
            nc.scalar.copy(outB[64:65, :], pc[0:1, _ONES:_ONES + 84])
            nc.vector.tensor_scalar_mul(tmpB[:], out0T, pm[0:64, 4:5])
            nc.vector.scalar_tensor_tensor(outB[0:64, 0:64], out1T,
                                           pm[0:64, 5:6], tmpB[:],
                                           MUL, ADD)

            tab_mms(1)

            tabs = pool.tile([128, 128], bf16)
            nc.vector.tensor_copy(tabs[:, 0:64], tabP[:, 0:64])
            nc.scalar.copy(tabs[:, 64:128], tabP[:, 64:128])

            # ---- attn halves via 3 accumulating matmuls [64, 168] ----
            attnP = pp.tile([64, 168], f32, tag="attn")
            nc.tensor.matmul(attnP[:], tabs[:, 0:64], pc[:, _CG:_CG + 168],
                             start=True, stop=False)
            nc.tensor.matmul(attnP[:], tabs[:, 64:128], pc[:, _CS:_CS + 168],
                             start=False, stop=False)
            nc.tensor.matmul(attnP[:], pc[0:2, _BIAS:_BIAS + 64],
                             pc[0:2, _CNT:_CNT + 168], start=False, stop=True)
            attns = pool.tile([64, 168], bf16)
            nc.vector.tensor_copy(attns[:, 0:84], attnP[:, 0:84])
            nc.scalar.copy(attns[:, 84:168], attnP[:, 84:168])

            # ---- decode og/ob in parallel psum banks ----
            deca = pc[0:64, _DECA:_DECA + V]
            decb2 = pc[0:65, _DECB:_DECB + V]
            ogP = pp.tile([84, V], f32, tag="og")
            obP = pp.tile([84, V], f32, tag="ob")
            nc.tensor.matmul(ogP[:], outG[:], decb2, start=True, stop=False)
            nc.tensor.matmul(ogP[:], attns[0:64, 0:84], deca, start=False, stop=True)
            nc.tensor.matmul(obP[:], outB[:], decb2, start=True, stop=False)
            nc.tensor.matmul(obP[:], attns[0:64, 84:168], deca, start=False, stop=True)
            # separate full-row tiles: HWDGE only spreads a DMA across the 16
            # SDMA engines when the SBUF side is flat-contiguous (offset 0,
            # whole partition rows) - partial-row reads pin to one engine.
            og_s = pool.tile([84, V], bf16)
            ob_s = pool.tile([84, V], bf16)
            nc.vector.tensor_copy(og_s[:], ogP[:])
            nc.scalar.copy(ob_s[:], obP[:])
            og_b = og_s[:]
            ogT = og_s[:].tensor
            obT = ob_s[:].tensor

            # ---- chunks 21:128 via selection matmuls (partition regroup) --
            pb01 = pp.tile([NB, 260], f32, tag="pb01")
            pb23 = pp.tile([NB, 260], f32, tag="pb23")
            for j, (dst, off) in enumerate(
                    [(pb01, 0), (pb01, V), (pb23, 0), (pb23, V)]):
                nc.tensor.matmul(dst[:, off:off + V],
                                 pc[0:84, _UB + 107 * j:_UB + 107 * j + 107],
                                 og_b, start=True, stop=True)
            ogB = pool.tile([NB, CW], bf16)
            nc.vector.tensor_copy(ogB[:, 0:260], pb01[:])
            nc.scalar.copy(ogB[:, 260:CW], pb23[:])
            ogBT = ogB[:].tensor

            # ---- output DMAs ----
            # tail (chunks 21:128) of ALL 8 slots (slot-0 tail is generic)
            nc.sync.dma_start(
                out=bass.AP(d_out, 84 * V, [[CW, NB], [SLOT, 8], [1, CW]]),
                in_=bass.AP(ogBT, 0, [[CW, NB], [0, 8], [1, CW]]))
            # heads: rows 0:84 map 1:1 to og/ob rows -> DMA straight from them
            nc.scalar.dma_start(
                out=bass.AP(d_out, SLOT, [[V, 84], [SLOT, 7], [1, V]]),
                in_=bass.AP(ogT, 0, [[V, 84], [0, 7], [1, V]]))
            nc.scalar.dma_start(
                out=bass.AP(d_out, 0, [[V, 84], [1, V]]),
                in_=bass.AP(obT, 0, [[V, 84], [1, V]]))

    nc.compile()
    return nc


def _get_nc():
    if "nc" not in _NC_CACHE:
        _NC_CACHE["nc"] = _build_nc()
    return _NC_CACHE["nc"]


def _host_reference_fallback(inputs):
    """Pure-numpy replica of the reference for steps != 512 (never hit with the
    canonical setup_inputs, which fixes lengths = 512)."""
    emb = inputs["emb"]; L = 2
    Ls = np.asarray(inputs["lengths"]); steps = int(Ls.max()); batch = inputs["inputs"].shape[0]
    layers = [(inputs["Wih0"], inputs["bih0"], inputs["bhh0"]),
              (inputs["Wih1"], inputs["bih1"], inputs["bhh1"])]
    sig = lambda z: 1.0 / (1.0 + np.exp(-z))

    def step(x):
        hs, cs = [], []
        inp = x
        for (Wih, bih, bhh) in layers:
            g = inp @ Wih.T + bih + bhh
            i, f, gg, o = np.split(g, 4, axis=-1)
            c = sig(i) * np.tanh(gg)
            h = sig(o) * np.tanh(c)
            hs.append(h); cs.append(c); inp = h
        return inp.astype(np.float32), np.stack(hs).astype(np.float32), np.stack(cs).astype(np.float32)

    x0 = emb[inputs["inputs"][:, 0]]
    x1 = emb[inputs["inputs"][:, 1]]
    out0, h0, c0 = step(x0)
    out1, h1, c1 = step(x1)
    outputs = np.concatenate(
        [out0[None], np.broadcast_to(out1[None], (steps - 1, batch, H))], 0
    ).reshape(batch, steps, H)
    h_steps = np.concatenate(
        [h0, np.broadcast_to(h1[None], (steps - 1, L, batch, H)).reshape((steps - 1) * L, batch, H)], 0
    ).reshape(batch, steps, L * H)
    c_steps = np.concatenate(
        [c0, np.broadcast_to(c1[None], (steps - 1, L, batch, H)).reshape((steps - 1) * L, batch, H)], 0
    ).reshape(batch, steps, L * H)
    Wh = h_steps @ inputs["Whw"].T + inputs["Whb"]
    Wc = c_steps @ inputs["Wcw"].T + inputs["Wcb"]
    idx = np.arange(steps)[:, None] + np.arange(A)[None, :] - A
    valid = idx >= 0
    win = np.where(valid[None, :, :, None], Wh[:, np.clip(idx, 0, None)], 0.0)
    att = win + Wc[:, :, None, :]
    attn = att.mean(axis=2)
    concat_h = np.concatenate([attn, outputs], axis=2)
    outs = concat_h @ inputs["decw"].T + inputs["decb"]
    bi, ti = np.nonzero(np.arange(steps)[None, :] < (Ls[:, None] - 1))
    return outs[bi, ti].reshape(-1, V).astype(np.float32)


def _g_generic(t):
    u = t % 64
    return 32 + u if u < 32 else 96 + (u - 32)


def _g_b0(t):
    if t >= 64:
        return 32 + (t - 64)
    return t if t < 32 else 64 + (t - 32)


def _window_counts(gfun):
    C = np.zeros((128, 84), np.float32)
    for t in range(84):
        for s in range(max(0, t - 20), t):
            C[gfun(s), t] += 1.0
    return C


def _selection(gfun):
    C = np.zeros((128, 84), np.float32)
    for t in range(84):
        C[gfun(t), t] = 1.0
    return C


def _pack_inputs(inputs):
    import ml_dtypes
    f32 = np.float32
    bf = ml_dtypes.bfloat16
    emb = inputs["emb"].astype(f32)
    idx0 = np.asarray(inputs["inputs"][:, 0]).astype(np.int64)
    idx1 = np.asarray(inputs["inputs"][:, 1]).astype(np.int64)

    def gates_pack(Wih):
        W = np.asarray(Wih, dtype=f32)
        return np.concatenate([W[0:H], W[2 * H:3 * H], W[3 * H:4 * H]], axis=0).T

    pa = np.zeros((64, _PAW), f32)
    pa[:, _XS:_XS + 64] = emb[idx0].T
    pa[:, _XS + 64:_XS + 128] = emb[idx1].T
    pa[:, _WIH0:_WIH0 + 192] = gates_pack(inputs["Wih0"])
    pa[:, _WIH1:_WIH1 + 192] = gates_pack(inputs["Wih1"])
    # hcat/ccat hold 2h/2c -> fold the 0.5 into every consumer here
    Whw = np.asarray(inputs["Whw"], f32)
    Wcw = np.asarray(inputs["Wcw"], f32)
    pa[:, _WHW:_WHW + 64] = Whw[:, 0:H].T / (2 * A)
    pa[:, _WHW + 64:_WHW + 128] = Whw[:, H:2 * H].T / (2 * A)
    pa[:, _WCW:_WCW + 64] = Wcw[:, 0:H].T / 2
    pa[:, _WCW + 64:_WCW + 128] = Wcw[:, H:2 * H].T / 2
    pa = pa.astype(bf)

    # gate biases packed as b_i/2, b_g, b_o/2 (tanh-only formulation)
    pmb = np.zeros((128, _PMW), f32)
    b0 = np.asarray(inputs["bih0"], f32) + np.asarray(inputs["bhh0"], f32)
    b1 = np.asarray(inputs["bih1"], f32) + np.asarray(inputs["bhh1"], f32)
    pmb[0:64, 0] = b0[0:H] / 2
    pmb[64:128, 0] = b0[2 * H:3 * H]
    pmb[0:64, 1] = b0[3 * H:4 * H] / 2
    pmb[0:64, 2] = b1[0:H] / 2
    pmb[64:128, 2] = b1[2 * H:3 * H]
    pmb[0:64, 3] = b1[3 * H:4 * H] / 2
    pmb[0:64, 5] = 1.0  # 1-mvec (generic cores); core 0 overridden below

    decw = np.asarray(inputs["decw"], f32)
    CGh = _window_counts(_g_generic)
    CZh = _window_counts(_g_b0)
    CSelG = _selection(_g_generic)
    CSelZ = _selection(_g_b0)
    cnt = np.minimum(np.arange(84), 20).astype(f32)

    def packc_for(core):
        p = np.zeros((128, _PCW), f32)
        p[:, _CG:_CG + 84] = CGh
        p[:, _CG + 84:_CG + 168] = CZh if core == 0 else CGh
        p[:, _CS:_CS + 84] = CSelG
        p[:, _CS + 84:_CS + 168] = CSelZ if core == 0 else CSelG
        p[0, _BIAS:_BIAS + 64] = np.asarray(inputs["Whb"], f32) / A
        p[1, _BIAS:_BIAS + 64] = np.asarray(inputs["Wcb"], f32)
        p[0, _CNT:_CNT + 84] = cnt
        p[0, _CNT + 84:_CNT + 168] = cnt
        p[1, _CNT:_CNT + 168] = 1.0
        p[0, _ONES:_ONES + 84] = 1.0
        for j in range(4):
            for q in range(NB):
                p[20 + (4 * q) % 64 + j, _UB + 107 * j + q] = 1.0
        p[0:64, _DECA:_DECA + V] = decw[:, 0:H].T
        p[0:64, _DECB:_DECB + V] = decw[:, H:2 * H].T / 2  # outG/outB hold 2h
        p[64, _DECB:_DECB + V] = np.asarray(inputs["decb"], f32)
        return p.astype(bf)

    pc0 = packc_for(0)
    pcg = packc_for(1)
    pm0 = pmb.copy()
    pm0[0:64, 4] = 1.0
    pm0[0:64, 5] = 0.0
    in_maps = []
    for core in range(NCORES):
        in_maps.append({"packa": pa,
                        "packm": pm0 if core == 0 else pmb,
                        "packc": pc0 if core == 0 else pcg})
    return in_maps


def kernel(**inputs):
    inputs = {k: np.asarray(v) for k, v in inputs.items()}
    Ls = np.asarray(inputs["lengths"]).astype(np.int64)
    steps = int(Ls.max())
    if steps != S or inputs["inputs"].shape != (B, S):
        return _host_reference_fallback(inputs)

    from concourse.bass_utils import run_bass_kernel_spmd

    in_maps = _pack_inputs(inputs)
    nc = _get_nc()
    res = run_bass_kernel_spmd(nc, in_maps, core_ids=list(range(NCORES)))
    outs = np.concatenate(
        [r["out"].astype(np.float32).reshape(BPC, S, V) for r in res.results],
        axis=0)  # [64,512,130]

    bi, ti = np.nonzero(np.arange(steps)[None, :] < (Ls[:, None] - 1))
    return np.ascontiguousarray(outs[bi, ti].reshape(-1, V))


# revision 20
# speedup vs baseline: 2.7958x; 2.4603x over previous
"""Trainium2 Bass kernel for nn_Melody_RNN (B=64, S=512, A=20, V=130, E=H=64, L=2).

Structure exploited (all implied by the reference's exact semantics):
  * Only embedding rows for inputs[:,0] / inputs[:,1] are used; the LSTM runs
    exactly 2 timesteps (zero initial state, so the forget gate is dead).
  * The torch cat+view memory reinterpretations make every output row depend
    only on s%64 (plus batch-0 specials for s<84), so the unique content is
    og[84,130] (generic) + ob[84,130] (batch-0 head, core 0 only).
  * The attention-mask bug makes softmax exactly uniform.

Kernel v9 notes (vs v8, 28.5us measured):
  * Middle section fully matmul-ized: tables are computed TRANSPOSED
    (tabT[j=128,h]) so the sliding-window-20 sum, the Wc column selection
    and the Whb/Wcb bias terms all become one 3-matmul psum accumulation
    against host-packed count/selection/bias matrices [128,168]
    (cols 0:84 generic, 84:168 batch-0 variant).  Replaces the shift-add
    window tree + 12 bias-add copies of v8.
  * Batch-0 blend for the attention half is gone: core 0 simply gets
    different packed constants (CZh/CSelZ) than cores 1-7 (CGh/CSelG).
    Only the outputs-half keeps a 2-op early blend via mvec.
  * og/ob decoded in parallel psum banks into one ogob[84,260] tile.
    Slot rows 0:84 (chunks 0:21) map 1:1 to og rows, so the head DMAs
    read ogob directly (260B descriptors, half-rate but tiny);
    only chunks 21:128 need the partition-regrouping selection matmuls.
  * Output: big DMA covers chunks 21:128 of ALL 8 slots (slot-0 tail is
    generic), column-split into two pipelined DMAs; head7 writes generic
    rows 0:84 into slots 1-7; head0 writes ob rows into slot 0.
  * Single ACT table load: one sigmoid dummy first on the ACT queue makes
    the compiler pick act-func-set 2 (sigmoid_and_others) which also
    contains tanh/identity/copy.

SPMD: 8 cores, identical program; per-core input differs in the window/
selection constant columns 84:168 of packc and the mvec blend columns.
"""

import sys
import numpy as np

if "/root/.axon_site/_ro/trn_rl_repo" not in sys.path:
    sys.path.insert(0, "/root/.axon_site/_ro/trn_rl_repo")

B, S, A = 64, 512, 20
V, E, H = 130, 64, 64
NCORES = 8
BPC = B // NCORES  # batches (slots) per core

# packa column layout (bf16, 64 partitions)
_XS = 0            # [64, 128] = [x0T | x1T]
_WIH0 = 128        # [64, 192] gates (i,g,o)
_WIH1 = 320        # [64, 192]
_WHW = 512         # [64, 128] = [Whw[:, :64].T | Whw[:, 64:].T] pre-scaled 1/A
_WCW = 640         # [64, 128]
_PAW = 768

# packm (f32, 128 partitions): LSTM gate biases + blend scalars
# cols: 0: l0 [i|g], 1: l0 o, 2: l1 [i|g], 3: l1 o, 4: mvec, 5: 1-mvec
_PMW = 8

# packc column layout (bf16, 128 partitions)
_CG = 0            # [128, 168] window counts: generic | per-core (CZh on core 0)
_CS = 168          # [128, 168] Wc selection: generic | per-core
_BIAS = 336        # [2, 64] rows: Whb/A, Wcb
_CNT = 400         # [2, 168] rows: min(t,20), ones
_ONES = 568        # [1, 84] ones (decb row source for outG/outB row 64)
_UB = 652          # 4 x [84, 128] chunk-selection for sub-row j (cols 0:21 = head)
_DECA = 1164       # [64, 130] decw[:, 0:64].T
_DECB = 1294       # [65, 130] decw[:, 64:128].T + decb row
_PCW = 1424

SLOT = S * V       # elements per output slot (66560)
CW = 4 * V         # chunk width in elements (520)
NCHUNK = 128       # 4-row chunks per slot


def _chunk_base(m):
    """Slot rows 4m..4m+4 == og[c .. c+4] (generic slots)."""
    return 4 * m if m <= 20 else 20 + (4 * m - 84) % 64

_NC_CACHE = {}


def _build_nc():
    import concourse.bass as bass
    import concourse.bacc as bacc
    import concourse.mybir as mybir
    from concourse.tile import TileContext

    f32 = mybir.dt.float32
    bf16 = mybir.dt.bfloat16
    AF = mybir.ActivationFunctionType

    nc = bacc.Bacc("TRN2", target_bir_lowering=False, debug=False)

    d_pa = nc.dram_tensor("packa", [64, _PAW], bf16, kind="ExternalInput")
    d_pm = nc.dram_tensor("packm", [128, _PMW], f32, kind="ExternalInput")
    d_pc = nc.dram_tensor("packc", [128, _PCW], bf16, kind="ExternalInput")
    d_out = nc.dram_tensor("out", [BPC * S, V], bf16, kind="ExternalOutput")

    with TileContext(nc) as tc:
        with (
            tc.tile_pool(name="sbuf", bufs=1) as pool,
            tc.tile_pool(name="psum", bufs=1, space="PSUM") as pp,
        ):
            # ---- input loads: sync queue gets the LSTM-critical columns ----
            pa = pool.tile([64, _PAW], bf16)
            pm = pool.tile([128, _PMW], f32)
            pc = pool.tile([128, _PCW], bf16)
            nc.sync.dma_start(out=pa[:, 0:_WIH1], in_=d_pa[:, 0:_WIH1])
            nc.scalar.dma_start(out=pm[:], in_=d_pm[:])
            nc.sync.dma_start(out=pa[:, _WIH1:_PAW], in_=d_pa[:, _WIH1:_PAW])
            nc.scalar.dma_start(out=pc[:], in_=d_pc[:])

            xs = pa[:, _XS:_XS + 128]
            wih0 = pa[:, _WIH0:_WIH0 + 192]
            wih1 = pa[:, _WIH1:_WIH1 + 192]
            whw = pa[:, _WHW:_WHW + 128]
            wcw = pa[:, _WCW:_WCW + 128]
            biasp = pm[:, 0:8]

            # ---- LSTM: both steps batched; hcat/ccat hold 2h / 2c
            #      (tanh-only formulation: sigma(x) = (tanh(x/2)+1)/2, the
            #      x2 folds into consumers on the host).  Tanh-only keeps the
            #      act-table pass on ONE hoisted set-0 load.
            #      cols [l0s0(b) | l0s1(b) | l1s0(b) | l1s1(b)]
            hcat = pool.tile([H, 256], bf16)
            ccat = pool.tile([H, 256], bf16)
            MUL = mybir.AluOpType.mult
            ADD = mybir.AluOpType.add

            def lstm_layer(rhsT, wp, bc, dst_off, tag, insc):
                # insc: 1.0 when rhsT holds true inputs, 0.5 when it holds 2h
                ps0 = pp.tile([128, 128], f32, tag="gates")   # [i|g]
                ps1 = pp.tile([64, 128], f32, tag="gateso")   # [o]
                nc.tensor.matmul(ps0[:], wp[:, 0:128], rhsT, start=True, stop=True)
                nc.tensor.matmul(ps1[:], wp[:, 128:192], rhsT, start=True, stop=True)
                tanh_i = pool.tile([H, 128], bf16, tag=f"ti{tag}")
                tanh_g = pool.tile([H, 128], bf16, tag=f"tg{tag}")
                tanh_o = pool.tile([H, 128], bf16, tag=f"to{tag}")
                tanh_c = pool.tile([H, 128], bf16, tag=f"tc{tag}")
                cc = ccat[:, dst_off:dst_off + 128]  # holds 2c
                hh = hcat[:, dst_off:dst_off + 128]  # holds 2h
                # biases packed as b_i/2, b_g, b_o/2
                nc.scalar.activation(tanh_g[:], ps0[64:128, :], AF.Tanh,
                                     bias=biasp[64:128, bc:bc + 1], scale=insc)
                nc.scalar.activation(tanh_i[:], ps0[0:64, :], AF.Tanh,
                                     bias=biasp[0:64, bc:bc + 1], scale=0.5 * insc)
                # 2c = (tanh_i + 1) * tanh_g
                nc.vector.scalar_tensor_tensor(cc, tanh_i[:], 1.0, tanh_g[:],
                                               ADD, MUL)
                nc.scalar.activation(tanh_o[:], ps1[0:64, :], AF.Tanh,
                                     bias=biasp[0:64, bc + 1:bc + 2],
                                     scale=0.5 * insc)
                nc.scalar.activation(tanh_c[:], cc, AF.Tanh, scale=0.5)
                # 2h = (tanh_o + 1) * tanh_c
                nc.vector.scalar_tensor_tensor(hh, tanh_o[:], 1.0, tanh_c[:],
                                               ADD, MUL)

            lstm_layer(xs, wih0, 0, 0, "l0", 1.0)
            lstm_layer(hcat[:, 0:128], wih1, 2, 128, "l1", 0.5)
            out0T = hcat[:, 128:192]   # l1 s0
            out1T = hcat[:, 192:256]   # l1 s1

            # ---- transposed tables tabT[j=128, 0:64]=h, [.., 64:128]=c ----
            # layer-0 half emitted after the l1 gates matmuls so the PE runs
            # it during the l1 activation chain.
            # bank is allocated wider: cols 128:388 recycled for the slot-0
            # head image sub-rows 2/3
            tabW = pp.tile([128, 388], f32, tag="tab")
            tabP = tabW[:, 0:128]

            def tab_mms(L):
                lo = 128 * L
                for src, w, coff in ((hcat, whw, 0), (ccat, wcw, 64)):
                    nc.tensor.matmul(tabP[64 * L:64 * L + 64, coff:coff + 64],
                                     src[:, lo + 0:lo + 128:2], w[:, 0:64],
                                     start=True, stop=False)
                    nc.tensor.matmul(tabP[64 * L:64 * L + 64, coff:coff + 64],
                                     src[:, lo + 1:lo + 128:2], w[:, 64:128],
                                     start=False, stop=True)

            tab_mms(0)

            # ---- outputs-half tiles (early; gpsimd/vector are idle) ----
            outG = pool.tile([65, 84], bf16)
            outB = pool.tile([65, 84], bf16)
            tmpB = pool.tile([64, 64], bf16)
            nc.gpsimd.tensor_copy(outG[0:64, 0:64], out1T)
            nc.gpsimd.tensor_copy(outG[0:64, 64:84], out1T[:, 0:20])
            nc.gpsimd.tensor_copy(outB[0:64, 64:84], out1T[:, 0:20])
            nc.scalar.copy(outG[64:65, :], pc[0:1, _ONES:_ONES + 84])
            nc.scalar.copy(outB[64:65, :], pc[0:1, _ONES:_ONES + 84])
            nc.vector.tensor_scalar_mul(tmpB[:], out0T, pm[0:64, 4:5])
            nc.vector.scalar_tensor_tensor(outB[0:64, 0:64], out1T,
                                           pm[0:64, 5:6], tmpB[:],
                                           MUL, ADD)

            tab_mms(1)

            tabs = pool.tile([128, 128], bf16)
            nc.vector.tensor_copy(tabs[:, 0:64], tabP[:, 0:64])
            nc.scalar.copy(tabs[:, 64:128], tabP[:, 64:128])

            # ---- attn halves via 3 accumulating matmuls [64, 168] ----
            # (bank is allocated wider: cols 168:428 are recycled later for
            # the slot-0 head image sub-rows 0/1)
            attnW = pp.tile([64, 448], f32, tag="attn")
            attnP = attnW[:, 0:168]
            nc.tensor.matmul(attnP[:], tabs[:, 0:64], pc[:, _CG:_CG + 168],
                             start=True, stop=False)
            nc.tensor.matmul(attnP[:], tabs[:, 64:128], pc[:, _CS:_CS + 168],
                             start=False, stop=False)
            nc.tensor.matmul(attnP[:], pc[0:2, _BIAS:_BIAS + 64],
                             pc[0:2, _CNT:_CNT + 168], start=False, stop=True)
            attns = pool.tile([64, 168], bf16)
            nc.vector.tensor_copy(attns[:, 0:84], attnP[:, 0:84])
            nc.scalar.copy(attns[:, 84:168], attnP[:, 84:168])

            # ---- decode og/ob in parallel psum banks ----
            deca = pc[0:64, _DECA:_DECA + V]
            decb2 = pc[0:65, _DECB:_DECB + V]
            ogP = pp.tile([84, V], f32, tag="og")
            obP = pp.tile([84, V], f32, tag="ob")
            nc.tensor.matmul(ogP[:], outG[:], decb2, start=True, stop=False)
            nc.tensor.matmul(ogP[:], attns[0:64, 0:84], deca, start=False, stop=True)
            nc.tensor.matmul(obP[:], outB[:], decb2, start=True, stop=False)
            nc.tensor.matmul(obP[:], attns[0:64, 84:168], deca, start=False, stop=True)
            og_s = pool.tile([84, V], bf16)
            ob_s = pool.tile([84, V], bf16)
            nc.vector.tensor_copy(og_s[:], ogP[:])
            nc.scalar.copy(ob_s[:], obP[:])
            og_b = og_s[:]

            # ---- full generic slot image via selection matmuls ----
            pb01 = pp.tile([NCHUNK, 260], f32, tag="pb01")
            pb23 = pp.tile([NCHUNK, 260], f32, tag="pb23")
            for j, (dst, off) in enumerate(
                    [(pb01, 0), (pb01, V), (pb23, 0), (pb23, V)]):
                nc.tensor.matmul(dst[:, off:off + V],
                                 pc[0:84, _UB + 128 * j:_UB + 128 * j + 128],
                                 og_b, start=True, stop=True)
            og8s = pool.tile([NCHUNK, CW], bf16)
            nc.vector.tensor_copy(og8s[:, 0:260], pb01[:])
            nc.scalar.copy(og8s[:, 260:CW], pb23[:])
            og8sT = og8s[:].tensor

            # ---- slot-0 head image (ob chunks 0:21) into recycled psum ----
            # attn/tab banks are dead by now; their spare columns hold the
            # [21, 130] sub-row blocks so the casts read bank-local data.
            for j, (dst, off) in enumerate(
                    [(attnW, 168), (attnW, 298), (tabW, 128), (tabW, 258)]):
                nc.tensor.matmul(dst[0:21, off:off + V],
                                 pc[0:84, _UB + 128 * j:_UB + 128 * j + 21],
                                 ob_s[:], start=True, stop=True)
            obA8 = pool.tile([21, CW], bf16)
            nc.vector.tensor_copy(obA8[:, 0:260], attnW[0:21, 168:428])
            nc.scalar.copy(obA8[:, 260:CW], tabW[0:21, 128:388])
            obA8T = obA8[:].tensor

            # ---- output DMAs (both on the sync queue, in this order) ----
            # 1) full image to ALL 8 slots; 2) slot-0 head overwrite with ob.
            # Same queue + identical partition->engine mapping (partitions
            # 0:21) makes the overwrite land after the generic head on every
            # SDMA engine.
            nc.sync.dma_start(
                out=bass.AP(d_out, 0, [[CW, NCHUNK], [SLOT, 8], [1, CW]]),
                in_=bass.AP(og8sT, 0, [[CW, NCHUNK], [0, 8], [1, CW]]))
            nc.sync.dma_start(
                out=bass.AP(d_out, 0, [[CW, 21], [1, CW]]),
                in_=bass.AP(obA8T, 0, [[CW, 21], [1, CW]]))

    nc.compile()

    # The slot-0 head overwrite relies on same-queue FIFO ordering: the full
    # 8-slot image DMA must precede the ob head DMA in the SP stream.
    sp_out_sizes = []
    for func in nc.m.functions:
        for block in func.blocks:
            for inst in block.instructions:
                if type(inst).__name__ != "InstDMACopy":
                    continue
                if str(getattr(inst, "engine", "")) != "EngineType.SP":
                    continue
                outs = getattr(inst, "outs", [])
                if outs and "out" in str(getattr(outs[0], "memref", "")):
                    sp_out_sizes.append(sum(
                        np.prod([c for _, c in arg.ap])
                        for arg in outs if hasattr(arg, "ap")))
    assert len(sp_out_sizes) == 2 and sp_out_sizes[0] > sp_out_sizes[1], (
        f"output DMA order broken: {sp_out_sizes}")
    return nc


def _get_nc():
    if "nc" not in _NC_CACHE:
        _NC_CACHE["nc"] = _build_nc()
    return _NC_CACHE["nc"]


def _host_reference_fallback(inputs):
    """Pure-numpy replica of the reference for steps != 512 (never hit with the
    canonical setup_inputs, which fixes lengths = 512)."""
    emb = inputs["emb"]; L = 2
    Ls = np.asarray(inputs["lengths"]); steps = int(Ls.max()); batch = inputs["inputs"].shape[0]
    layers = [(inputs["Wih0"], inputs["bih0"], inputs["bhh0"]),
              (inputs["Wih1"], inputs["bih1"], inputs["bhh1"])]
    sig = lambda z: 1.0 / (1.0 + np.exp(-z))

    def step(x):
        hs, cs = [], []
        inp = x
        for (Wih, bih, bhh) in layers:
            g = inp @ Wih.T + bih + bhh
            i, f, gg, o = np.split(g, 4, axis=-1)
            c = sig(i) * np.tanh(gg)
            h = sig(o) * np.tanh(c)
            hs.append(h); cs.append(c); inp = h
        return inp.astype(np.float32), np.stack(hs).astype(np.float32), np.stack(cs).astype(np.float32)

    x0 = emb[inputs["inputs"][:, 0]]
    x1 = emb[inputs["inputs"][:, 1]]
    out0, h0, c0 = step(x0)
    out1, h1, c1 = step(x1)
    outputs = np.concatenate(
        [out0[None], np.broadcast_to(out1[None], (steps - 1, batch, H))], 0
    ).reshape(batch, steps, H)
    h_steps = np.concatenate(
        [h0, np.broadcast_to(h1[None], (steps - 1, L, batch, H)).reshape((steps - 1) * L, batch, H)], 0
    ).reshape(batch, steps, L * H)
    c_steps = np.concatenate(
        [c0, np.broadcast_to(c1[None], (steps - 1, L, batch, H)).reshape((steps - 1) * L, batch, H)], 0
    ).reshape(batch, steps, L * H)
    Wh = h_steps @ inputs["Whw"].T + inputs["Whb"]
    Wc = c_steps @ inputs["Wcw"].T + inputs["Wcb"]
    idx = np.arange(steps)[:, None] + np.arange(A)[None, :] - A
    valid = idx >= 0
    win = np.where(valid[None, :, :, None], Wh[:, np.clip(idx, 0, None)], 0.0)
    att = win + Wc[:, :, None, :]
    attn = att.mean(axis=2)
    concat_h = np.concatenate([attn, outputs], axis=2)
    outs = concat_h @ inputs["decw"].T + inputs["decb"]
    bi, ti = np.nonzero(np.arange(steps)[None, :] < (Ls[:, None] - 1))
    return outs[bi, ti].reshape(-1, V).astype(np.float32)


def _g_generic(t):
    u = t % 64
    return 32 + u if u < 32 else 96 + (u - 32)


def _g_b0(t):
    if t >= 64:
        return 32 + (t - 64)
    return t if t < 32 else 64 + (t - 32)


def _window_counts(gfun):
    C = np.zeros((128, 84), np.float32)
    for t in range(84):
        for s in range(max(0, t - 20), t):
            C[gfun(s), t] += 1.0
    return C


def _selection(gfun):
    C = np.zeros((128, 84), np.float32)
    for t in range(84):
        C[gfun(t), t] = 1.0
    return C


def _pack_inputs(inputs):
    import ml_dtypes
    f32 = np.float32
    bf = ml_dtypes.bfloat16
    emb = inputs["emb"].astype(f32)
    idx0 = np.asarray(inputs["inputs"][:, 0]).astype(np.int64)
    idx1 = np.asarray(inputs["inputs"][:, 1]).astype(np.int64)

    def gates_pack(Wih):
        W = np.asarray(Wih, dtype=f32)
        return np.concatenate([W[0:H], W[2 * H:3 * H], W[3 * H:4 * H]], axis=0).T

    pa = np.zeros((64, _PAW), f32)
    pa[:, _XS:_XS + 64] = emb[idx0].T
    pa[:, _XS + 64:_XS + 128] = emb[idx1].T
    pa[:, _WIH0:_WIH0 + 192] = gates_pack(inputs["Wih0"])
    pa[:, _WIH1:_WIH1 + 192] = gates_pack(inputs["Wih1"])
    # hcat/ccat hold 2h/2c -> fold the 0.5 into every consumer here
    Whw = np.asarray(inputs["Whw"], f32)
    Wcw = np.asarray(inputs["Wcw"], f32)
    pa[:, _WHW:_WHW + 64] = Whw[:, 0:H].T / (2 * A)
    pa[:, _WHW + 64:_WHW + 128] = Whw[:, H:2 * H].T / (2 * A)
    pa[:, _WCW:_WCW + 64] = Wcw[:, 0:H].T / 2
    pa[:, _WCW + 64:_WCW + 128] = Wcw[:, H:2 * H].T / 2
    pa = pa.astype(bf)

    # gate biases packed as b_i/2, b_g, b_o/2 (tanh-only formulation)
    pmb = np.zeros((128, _PMW), f32)
    b0 = np.asarray(inputs["bih0"], f32) + np.asarray(inputs["bhh0"], f32)
    b1 = np.asarray(inputs["bih1"], f32) + np.asarray(inputs["bhh1"], f32)
    pmb[0:64, 0] = b0[0:H] / 2
    pmb[64:128, 0] = b0[2 * H:3 * H]
    pmb[0:64, 1] = b0[3 * H:4 * H] / 2
    pmb[0:64, 2] = b1[0:H] / 2
    pmb[64:128, 2] = b1[2 * H:3 * H]
    pmb[0:64, 3] = b1[3 * H:4 * H] / 2
    pmb[0:64, 5] = 1.0  # 1-mvec (generic cores); core 0 overridden below

    decw = np.asarray(inputs["decw"], f32)
    CGh = _window_counts(_g_generic)
    CZh = _window_counts(_g_b0)
    CSelG = _selection(_g_generic)
    CSelZ = _selection(_g_b0)
    cnt = np.minimum(np.arange(84), 20).astype(f32)

    def packc_for(core):
        p = np.zeros((128, _PCW), f32)
        p[:, _CG:_CG + 84] = CGh
        p[:, _CG + 84:_CG + 168] = CZh if core == 0 else CGh
        p[:, _CS:_CS + 84] = CSelG
        p[:, _CS + 84:_CS + 168] = CSelZ if core == 0 else CSelG
        p[0, _BIAS:_BIAS + 64] = np.asarray(inputs["Whb"], f32) / A
        p[1, _BIAS:_BIAS + 64] = np.asarray(inputs["Wcb"], f32)
        p[0, _CNT:_CNT + 84] = cnt
        p[0, _CNT + 84:_CNT + 168] = cnt
        p[1, _CNT:_CNT + 168] = 1.0
        p[0, _ONES:_ONES + 84] = 1.0
        for j in range(4):
            for m in range(NCHUNK):
                p[_chunk_base(m) + j, _UB + 128 * j + m] = 1.0
        p[0:64, _DECA:_DECA + V] = decw[:, 0:H].T
        p[0:64, _DECB:_DECB + V] = decw[:, H:2 * H].T / 2  # outG/outB hold 2h
        p[64, _DECB:_DECB + V] = np.asarray(inputs["decb"], f32)
        return p.astype(bf)

    pc0 = packc_for(0)
    pcg = packc_for(1)
    pm0 = pmb.copy()
    pm0[0:64, 4] = 1.0
    pm0[0:64, 5] = 0.0
    in_maps = []
    for core in range(NCORES):
        in_maps.append({"packa": pa,
                        "packm": pm0 if core == 0 else pmb,
                        "packc": pc0 if core == 0 else pcg})
    return in_maps


def kernel(**inputs):
    inputs = {k: np.asarray(v) for k, v in inputs.items()}
    Ls = np.asarray(inputs["lengths"]).astype(np.int64)
    steps = int(Ls.max())
    if steps != S or inputs["inputs"].shape != (B, S):
        return _host_reference_fallback(inputs)

    from concourse.bass_utils import run_bass_kernel_spmd

    in_maps = _pack_inputs(inputs)
    nc = _get_nc()
    res = run_bass_kernel_spmd(nc, in_maps, core_ids=list(range(NCORES)))
    outs = np.concatenate(
        [r["out"].astype(np.float32).reshape(BPC, S, V) for r in res.results],
        axis=0)  # [64,512,130]

    bi, ti = np.nonzero(np.arange(steps)[None, :] < (Ls[:, None] - 1))
    return np.ascontiguousarray(outs[bi, ti].reshape(-1, V))


# revision 32
# speedup vs baseline: 2.9291x; 1.0477x over previous
"""Trainium2 Bass kernel for nn_Melody_RNN (B=64, S=512, A=20, V=130, E=H=64, L=2).

Structure exploited (all implied by the reference's exact semantics):
  * Only embedding rows for inputs[:,0] / inputs[:,1] are used; the LSTM runs
    exactly 2 timesteps (zero initial state, so the forget gate is dead).
  * The torch cat+view memory reinterpretations make every output row depend
    only on s%64 (plus batch-0 specials for s<84), so the unique content is
    og[84,130] (generic) + ob[84,130] (batch-0 head, core 0 only).
  * The attention-mask bug makes softmax exactly uniform.

Kernel v9 notes (vs v8, 28.5us measured):
  * Middle section fully matmul-ized: tables are computed TRANSPOSED
    (tabT[j=128,h]) so the sliding-window-20 sum, the Wc column selection
    and the Whb/Wcb bias terms all become one 3-matmul psum accumulation
    against host-packed count/selection/bias matrices [128,168]
    (cols 0:84 generic, 84:168 batch-0 variant).  Replaces the shift-add
    window tree + 12 bias-add copies of v8.
  * Batch-0 blend for the attention half is gone: core 0 simply gets
    different packed constants (CZh/CSelZ) than cores 1-7 (CGh/CSelG).
    Only the outputs-half keeps a 2-op early blend via mvec.
  * og/ob decoded in parallel psum banks into one ogob[84,260] tile.
    Slot rows 0:84 (chunks 0:21) map 1:1 to og rows, so the head DMAs
    read ogob directly (260B descriptors, half-rate but tiny);
    only chunks 21:128 need the partition-regrouping selection matmuls.
  * Output: big DMA covers chunks 21:128 of ALL 8 slots (slot-0 tail is
    generic), column-split into two pipelined DMAs; head7 writes generic
    rows 0:84 into slots 1-7; head0 writes ob rows into slot 0.
  * Single ACT table load: one sigmoid dummy first on the ACT queue makes
    the compiler pick act-func-set 2 (sigmoid_and_others) which also
    contains tanh/identity/copy.

SPMD: 8 cores, identical program; per-core input differs in the window/
selection constant columns 84:168 of packc and the mvec blend columns.
"""

import sys
import numpy as np

if "/root/.axon_site/_ro/trn_rl_repo" not in sys.path:
    sys.path.insert(0, "/root/.axon_site/_ro/trn_rl_repo")

B, S, A = 64, 512, 20
V, E, H = 130, 64, 64
NCORES = 8
BPC = B // NCORES  # batches (slots) per core

# packa column layout (bf16, 64 partitions)
_XS = 0            # [64, 128] = [x0T | x1T]
_WIH0 = 128        # [64, 192] gates (i,g,o)
_WIH1 = 320        # [64, 192]
_WHW = 512         # [64, 128] = [Whw[:, :64].T | Whw[:, 64:].T] pre-scaled 1/A
_WCW = 640         # [64, 128]
_PAW = 768

# packm (f32, 128 partitions): LSTM gate biases + blend scalars
# cols: 0: l0 [i|g], 1: l0 o, 2: l1 [i|g], 3: l1 o, 4: mvec, 5: 1-mvec
_PMW = 8

# packc column layout (bf16, 128 partitions)
_CG = 0            # [128, 168] window counts: generic | per-core (CZh on core 0)
_CS = 168          # [128, 168] Wc selection: generic | per-core
_BIAS = 336        # [2, 64] rows: Whb/A, Wcb
_CNT = 400         # [2, 168] rows: min(t,20), ones
_ONES = 568        # [1, 84] ones (decb row source for outG/outB row 64)
_UB = 652          # 4 x [84, 128] chunk-selection for sub-row j (cols 0:21 = head)
_DECA = 1164       # [64, 130] decw[:, 0:64].T
_DECB = 1294       # [65, 130] decw[:, 64:128].T + decb row
_PCW = 1424

SLOT = S * V       # elements per output slot (66560)
CW = 4 * V         # chunk width in elements (520)
NCHUNK = 128       # 4-row chunks per slot


def _chunk_base(m):
    """Slot rows 4m..4m+4 == og[c .. c+4] (generic slots)."""
    return 4 * m if m <= 20 else 20 + (4 * m - 84) % 64

_NC_CACHE = {}


def _build_nc():
    import concourse.bass as bass
    import concourse.bacc as bacc
    import concourse.mybir as mybir
    from concourse.tile import TileContext

    f32 = mybir.dt.float32
    bf16 = mybir.dt.bfloat16
    AF = mybir.ActivationFunctionType

    nc = bacc.Bacc("TRN2", target_bir_lowering=False, debug=False)

    d_pa = nc.dram_tensor("packa", [64, _PAW], bf16, kind="ExternalInput")
    d_pm = nc.dram_tensor("packm", [128, _PMW], f32, kind="ExternalInput")
    d_pc = nc.dram_tensor("packc", [128, _PCW], bf16, kind="ExternalInput")
    d_out = nc.dram_tensor("out", [BPC * S, V], bf16, kind="ExternalOutput")

    with TileContext(nc) as tc:
        with (
            tc.tile_pool(name="sbuf", bufs=1) as pool,
            tc.tile_pool(name="psum", bufs=1, space="PSUM") as pp,
        ):
            # ---- input loads: sync queue gets the LSTM-critical columns ----
            pa = pool.tile([64, _PAW], bf16)
            pm = pool.tile([128, _PMW], f32)
            pc = pool.tile([128, _PCW], bf16)
            nc.sync.dma_start(out=pa[:, 0:_WIH1], in_=d_pa[:, 0:_WIH1])
            nc.scalar.dma_start(out=pm[:], in_=d_pm[:])
            nc.sync.dma_start(out=pa[:, _WIH1:_PAW], in_=d_pa[:, _WIH1:_PAW])
            nc.scalar.dma_start(out=pc[:], in_=d_pc[:])

            xs = pa[:, _XS:_XS + 128]
            wih0 = pa[:, _WIH0:_WIH0 + 192]
            wih1 = pa[:, _WIH1:_WIH1 + 192]
            whw = pa[:, _WHW:_WHW + 128]
            wcw = pa[:, _WCW:_WCW + 128]
            biasp = pm[:, 0:8]

            # ---- LSTM: both steps batched; hcat/ccat hold 2h / 2c
            #      (tanh-only formulation: sigma(x) = (tanh(x/2)+1)/2, the
            #      x2 folds into consumers on the host).  Tanh-only keeps the
            #      act-table pass on ONE hoisted set-0 load.
            #      cols [l0s0(b) | l0s1(b) | l1s0(b) | l1s1(b)]
            hcat = pool.tile([H, 256], bf16)
            ccat = pool.tile([H, 256], bf16)
            MUL = mybir.AluOpType.mult
            ADD = mybir.AluOpType.add

            # gates psum banks allocated full-width; the spare columns are
            # recycled for the og8sB selection-matmul outputs later.
            gatesW = pp.tile([128, 512], f32, tag="gates")
            gatesoW = pp.tile([64, 512], f32, tag="gateso")

            def lstm_layer(rhsT, wp, bc, dst_off, tag, insc):
                # insc: 1.0 when rhsT holds true inputs, 0.5 when it holds 2h
                # (i/o weight rows are host-prescaled by 0.5 so one scale
                # covers the whole [i|g] block)
                ps0 = gatesW[:, 0:128]   # [i|g]
                ps1 = gatesoW[:, 0:128]  # [o]
                nc.tensor.matmul(ps0, wp[:, 0:128], rhsT, start=True, stop=True)
                nc.tensor.matmul(ps1, wp[:, 128:192], rhsT, start=True, stop=True)
                tanh_i = pool.tile([H, 128], bf16, tag=f"ti{tag}")
                tanh_g = pool.tile([H, 128], bf16, tag=f"tg{tag}")
                tanh_o = pool.tile([H, 128], bf16, tag=f"to{tag}")
                tanh_c = pool.tile([H, 128], bf16, tag=f"tc{tag}")
                cc = ccat[:, dst_off:dst_off + 128]  # holds 2c
                hh = hcat[:, dst_off:dst_off + 128]  # holds 2h
                # biases packed as b_i/2, b_g, b_o/2 (i/o weight rows are
                # host-prescaled 0.5 so one scale fits all gates)
                nc.scalar.activation(tanh_g[:], ps0[64:128, 0:128], AF.Tanh,
                                     bias=biasp[64:128, bc:bc + 1], scale=insc)
                nc.scalar.activation(tanh_i[:], ps0[0:64, 0:128], AF.Tanh,
                                     bias=biasp[0:64, bc:bc + 1], scale=insc)
                # 2c = (tanh_i + 1) * tanh_g  (STT needs equal base partitions)
                nc.vector.scalar_tensor_tensor(cc, tanh_i[:], 1.0,
                                               tanh_g[:], ADD, MUL)
                nc.scalar.activation(tanh_o[:], ps1[0:64, :], AF.Tanh,
                                     bias=biasp[0:64, bc + 1:bc + 2],
                                     scale=insc)
                nc.scalar.activation(tanh_c[:], cc, AF.Tanh, scale=0.5)
                # 2h = (tanh_o + 1) * tanh_c
                nc.vector.scalar_tensor_tensor(hh, tanh_o[:], 1.0, tanh_c[:],
                                               ADD, MUL)

            lstm_layer(xs, wih0, 0, 0, "l0", 1.0)
            lstm_layer(hcat[:, 0:128], wih1, 2, 128, "l1", 0.5)
            out0T = hcat[:, 128:192]   # l1 s0
            out1T = hcat[:, 192:256]   # l1 s1

            # ---- transposed tables tabT[j=128, 0:64]=h, [.., 64:128]=c ----
            # layer-0 half emitted after the l1 gates matmuls so the PE runs
            # it during the l1 activation chain.
            # bank is allocated wider: cols 128:388 recycled for the slot-0
            # head image sub-rows 2/3
            tabW = pp.tile([128, 388], f32, tag="tab")
            tabP = tabW[:, 0:128]

            def tab_mms(L):
                lo = 128 * L
                for src, w, coff in ((hcat, whw, 0), (ccat, wcw, 64)):
                    nc.tensor.matmul(tabP[64 * L:64 * L + 64, coff:coff + 64],
                                     src[:, lo + 0:lo + 128:2], w[:, 0:64],
                                     start=True, stop=False)
                    nc.tensor.matmul(tabP[64 * L:64 * L + 64, coff:coff + 64],
                                     src[:, lo + 1:lo + 128:2], w[:, 64:128],
                                     start=False, stop=True)

            tab_mms(0)

            # ---- outputs-half tiles (early; gpsimd/vector are idle) ----
            outG = pool.tile([65, 84], bf16)
            outB = pool.tile([65, 84], bf16)
            tmpB = pool.tile([64, 64], bf16)
            nc.gpsimd.tensor_copy(outG[0:64, 0:64], out1T)
            nc.gpsimd.tensor_copy(outG[0:64, 64:84], out1T[:, 0:20])
            nc.gpsimd.tensor_copy(outB[0:64, 64:84], out1T[:, 0:20])
            nc.gpsimd.tensor_copy(outG[64:65, :], pc[0:1, _ONES:_ONES + 84])
            nc.gpsimd.tensor_copy(outB[64:65, :], pc[0:1, _ONES:_ONES + 84])
            nc.vector.tensor_scalar_mul(tmpB[:], out0T, pm[0:64, 4:5])
            nc.vector.scalar_tensor_tensor(outB[0:64, 0:64], out1T,
                                           pm[0:64, 5:6], tmpB[:],
                                           MUL, ADD)

            tab_mms(1)

            tabs = pool.tile([128, 128], bf16)
            nc.vector.tensor_copy(tabs[:], tabP[:])

            # ---- attn halves via 3 accumulating matmuls [64, 168] ----
            # (bank is allocated wider: cols 168:428 are recycled later for
            # the slot-0 head image sub-rows 0/1)
            attnW = pp.tile([64, 448], f32, tag="attn")
            attnP = attnW[:, 0:168]
            nc.tensor.matmul(attnP[:], tabs[:, 0:64], pc[:, _CG:_CG + 168],
                             start=True, stop=False)
            nc.tensor.matmul(attnP[:], tabs[:, 64:128], pc[:, _CS:_CS + 168],
                             start=False, stop=False)
            nc.tensor.matmul(attnP[:], pc[0:2, _BIAS:_BIAS + 64],
                             pc[0:2, _CNT:_CNT + 168], start=False, stop=True)
            attns = pool.tile([64, 168], bf16)
            nc.vector.tensor_copy(attns[:], attnP[:])

            # ---- decode og/ob in parallel psum banks ----
            deca = pc[0:64, _DECA:_DECA + V]
            decb2 = pc[0:65, _DECB:_DECB + V]
            ogP = pp.tile([84, V], f32, tag="og")
            obP = pp.tile([84, V], f32, tag="ob")
            nc.tensor.matmul(ogP[:], outG[:], decb2, start=True, stop=False)
            nc.tensor.matmul(ogP[:], attns[0:64, 0:84], deca, start=False, stop=True)
            nc.tensor.matmul(obP[:], outB[:], decb2, start=True, stop=False)
            nc.tensor.matmul(obP[:], attns[0:64, 84:168], deca, start=False, stop=True)
            og_s = pool.tile([84, V], bf16)
            ob_s = pool.tile([84, V], bf16)
            nc.vector.tensor_copy(og_s[:], ogP[:])
            nc.scalar.copy(ob_s[:], obP[:])
            og_b = og_s[:]

            # ---- generic slot image, split into two [64, 520] halves so
            #      the two big DMAs ride different queues.  The B-half psum
            #      lives in the recycled gates banks (partition regroup must
            #      land at partition 0 of its own bank). ----
            pbA01 = pp.tile([64, 260], f32, tag="pbA01")
            pbA23 = pp.tile([64, 260], f32, tag="pbA23")
            for j, (dst, off) in enumerate(
                    [(pbA01, 0), (pbA01, V), (pbA23, 0), (pbA23, V)]):
                nc.tensor.matmul(dst[:, off:off + V],
                                 pc[0:84, _UB + 128 * j:_UB + 128 * j + 64],
                                 og_b, start=True, stop=True)
            for j, (dst, off) in enumerate(
                    [(gatesW, 128), (gatesW, 258), (gatesoW, 128), (gatesoW, 258)]):
                nc.tensor.matmul(dst[0:64, off:off + V],
                                 pc[0:84, _UB + 128 * j + 64:_UB + 128 * j + 128],
                                 og_b, start=True, stop=True)
            og8sA = pool.tile([64, CW], bf16)
            og8sB = pool.tile([64, CW], bf16)
            nc.vector.tensor_copy(og8sA[:, 0:260], pbA01[:])
            nc.scalar.copy(og8sA[:, 260:CW], pbA23[:])
            nc.vector.tensor_copy(og8sB[:, 0:260], gatesW[0:64, 128:388])
            nc.scalar.copy(og8sB[:, 260:CW], gatesoW[0:64, 128:388])
            og8sAT = og8sA[:].tensor
            og8sBT = og8sB[:].tensor

            # ---- slot-0 head image (ob chunks 0:21) into recycled psum ----
            # attn/tab banks are dead by now; their spare columns hold the
            # [21, 130] sub-row blocks so the casts read bank-local data.
            for j, (dst, off) in enumerate(
                    [(attnW, 168), (attnW, 298), (tabW, 128), (tabW, 258)]):
                nc.tensor.matmul(dst[0:21, off:off + V],
                                 pc[0:84, _UB + 128 * j:_UB + 128 * j + 21],
                                 ob_s[:], start=True, stop=True)
            obA8 = pool.tile([21, CW], bf16)
            nc.vector.tensor_copy(obA8[:, 0:260], attnW[0:21, 168:428])
            nc.scalar.copy(obA8[:, 260:CW], tabW[0:21, 128:388])
            obA8T = obA8[:].tensor

            # ---- output DMAs ----
            # sync queue: A-half (chunks 0:64, all 8 slots) THEN the slot-0
            # head overwrite - same queue + identical partition->engine
            # mapping (partitions 0:21) makes the overwrite land after the
            # generic head on every SDMA engine.
            # scalar queue (parallel): B-half (chunks 64:128).
            nc.sync.dma_start(
                out=bass.AP(d_out, 0, [[CW, 64], [SLOT, 8], [1, CW]]),
                in_=bass.AP(og8sAT, 0, [[CW, 64], [0, 8], [1, CW]]))
            nc.sync.dma_start(
                out=bass.AP(d_out, 0, [[CW, 21], [1, CW]]),
                in_=bass.AP(obA8T, 0, [[CW, 21], [1, CW]]))
            nc.scalar.dma_start(
                out=bass.AP(d_out, 64 * CW, [[CW, 64], [SLOT, 8], [1, CW]]),
                in_=bass.AP(og8sBT, 0, [[CW, 64], [0, 8], [1, CW]]))

    nc.compile()

    # The slot-0 head overwrite relies on same-queue FIFO ordering: the full
    # 8-slot image DMA must precede the ob head DMA in the SP stream.
    sp_out_sizes = []
    for func in nc.m.functions:
        for block in func.blocks:
            for inst in block.instructions:
                if type(inst).__name__ != "InstDMACopy":
                    continue
                if str(getattr(inst, "engine", "")) != "EngineType.SP":
                    continue
                outs = getattr(inst, "outs", [])
                if outs and "out" in str(getattr(outs[0], "memref", "")):
                    sp_out_sizes.append(sum(
                        np.prod([c for _, c in arg.ap])
                        for arg in outs if hasattr(arg, "ap")))
    assert len(sp_out_sizes) == 2 and sp_out_sizes[0] > sp_out_sizes[1], (
        f"output DMA order broken: {sp_out_sizes}")
    return nc


def _get_nc():
    if "nc" not in _NC_CACHE:
        _NC_CACHE["nc"] = _build_nc()
    return _NC_CACHE["nc"]


def _host_reference_fallback(inputs):
    """Pure-numpy replica of the reference for steps != 512 (never hit with the
    canonical setup_inputs, which fixes lengths = 512)."""
    emb = inputs["emb"]; L = 2
    Ls = np.asarray(inputs["lengths"]); steps = int(Ls.max()); batch = inputs["inputs"].shape[0]
    layers = [(inputs["Wih0"], inputs["bih0"], inputs["bhh0"]),
              (inputs["Wih1"], inputs["bih1"], inputs["bhh1"])]
    sig = lambda z: 1.0 / (1.0 + np.exp(-z))

    def step(x):
        hs, cs = [], []
        inp = x
        for (Wih, bih, bhh) in layers:
            g = inp @ Wih.T + bih + bhh
            i, f, gg, o = np.split(g, 4, axis=-1)
            c = sig(i) * np.tanh(gg)
            h = sig(o) * np.tanh(c)
            hs.append(h); cs.append(c); inp = h
        return inp.astype(np.float32), np.stack(hs).astype(np.float32), np.stack(cs).astype(np.float32)

    x0 = emb[inputs["inputs"][:, 0]]
    x1 = emb[inputs["inputs"][:, 1]]
    out0, h0, c0 = step(x0)
    out1, h1, c1 = step(x1)
    outputs = np.concatenate(
        [out0[None], np.broadcast_to(out1[None], (steps - 1, batch, H))], 0
    ).reshape(batch, steps, H)
    h_steps = np.concatenate(
        [h0, np.broadcast_to(h1[None], (steps - 1, L, batch, H)).reshape((steps - 1) * L, batch, H)], 0
    ).reshape(batch, steps, L * H)
    c_steps = np.concatenate(
        [c0, np.broadcast_to(c1[None], (steps - 1, L, batch, H)).reshape((steps - 1) * L, batch, H)], 0
    ).reshape(batch, steps, L * H)
    Wh = h_steps @ inputs["Whw"].T + inputs["Whb"]
    Wc = c_steps @ inputs["Wcw"].T + inputs["Wcb"]
    idx = np.arange(steps)[:, None] + np.arange(A)[None, :] - A
    valid = idx >= 0
    win = np.where(valid[None, :, :, None], Wh[:, np.clip(idx, 0, None)], 0.0)
    att = win + Wc[:, :, None, :]
    attn = att.mean(axis=2)
    concat_h = np.concatenate([attn, outputs], axis=2)
    outs = concat_h @ inputs["decw"].T + inputs["decb"]
    bi, ti = np.nonzero(np.arange(steps)[None, :] < (Ls[:, None] - 1))
    return outs[bi, ti].reshape(-1, V).astype(np.float32)


def _g_generic(t):
    u = t % 64
    return 32 + u if u < 32 else 96 + (u - 32)


def _g_b0(t):
    if t >= 64:
        return 32 + (t - 64)
    return t if t < 32 else 64 + (t - 32)


def _window_counts(gfun):
    C = np.zeros((128, 84), np.float32)
    for t in range(84):
        for s in range(max(0, t - 20), t):
            C[gfun(s), t] += 1.0
    return C


def _selection(gfun):
    C = np.zeros((128, 84), np.float32)
    for t in range(84):
        C[gfun(t), t] = 1.0
    return C


def _pack_inputs(inputs):
    import ml_dtypes
    f32 = np.float32
    bf = ml_dtypes.bfloat16
    emb = inputs["emb"].astype(f32)
    idx0 = np.asarray(inputs["inputs"][:, 0]).astype(np.int64)
    idx1 = np.asarray(inputs["inputs"][:, 1]).astype(np.int64)

    def gates_pack(Wih):
        # rows [i/2, g, o/2]: the 0.5 on i/o makes one activation scale
        # cover the fused [i|g] block (tanh-only sigma formulation)
        W = np.asarray(Wih, dtype=f32)
        return np.concatenate([W[0:H] / 2, W[2 * H:3 * H], W[3 * H:4 * H] / 2],
                              axis=0).T

    pa = np.zeros((64, _PAW), f32)
    pa[:, _XS:_XS + 64] = emb[idx0].T
    pa[:, _XS + 64:_XS + 128] = emb[idx1].T
    pa[:, _WIH0:_WIH0 + 192] = gates_pack(inputs["Wih0"])
    pa[:, _WIH1:_WIH1 + 192] = gates_pack(inputs["Wih1"])
    # hcat/ccat hold 2h/2c -> fold the 0.5 into every consumer here
    Whw = np.asarray(inputs["Whw"], f32)
    Wcw = np.asarray(inputs["Wcw"], f32)
    pa[:, _WHW:_WHW + 64] = Whw[:, 0:H].T / (2 * A)
    pa[:, _WHW + 64:_WHW + 128] = Whw[:, H:2 * H].T / (2 * A)
    pa[:, _WCW:_WCW + 64] = Wcw[:, 0:H].T / 2
    pa[:, _WCW + 64:_WCW + 128] = Wcw[:, H:2 * H].T / 2
    pa = pa.astype(bf)

    # gate biases packed as b_i/2, b_g, b_o/2 (tanh-only formulation)
    pmb = np.zeros((128, _PMW), f32)
    b0 = np.asarray(inputs["bih0"], f32) + np.asarray(inputs["bhh0"], f32)
    b1 = np.asarray(inputs["bih1"], f32) + np.asarray(inputs["bhh1"], f32)
    pmb[0:64, 0] = b0[0:H] / 2
    pmb[64:128, 0] = b0[2 * H:3 * H]
    pmb[0:64, 1] = b0[3 * H:4 * H] / 2
    pmb[0:64, 2] = b1[0:H] / 2
    pmb[64:128, 2] = b1[2 * H:3 * H]
    pmb[0:64, 3] = b1[3 * H:4 * H] / 2
    pmb[0:64, 5] = 1.0  # 1-mvec (generic cores); core 0 overridden below

    decw = np.asarray(inputs["decw"], f32)
    CGh = _window_counts(_g_generic)
    CZh = _window_counts(_g_b0)
    CSelG = _selection(_g_generic)
    CSelZ = _selection(_g_b0)
    cnt = np.minimum(np.arange(84), 20).astype(f32)

    def packc_for(core):
        p = np.zeros((128, _PCW), f32)
        p[:, _CG:_CG + 84] = CGh
        p[:, _CG + 84:_CG + 168] = CZh if core == 0 else CGh
        p[:, _CS:_CS + 84] = CSelG
        p[:, _CS + 84:_CS + 168] = CSelZ if core == 0 else CSelG
        p[0, _BIAS:_BIAS + 64] = np.asarray(inputs["Whb"], f32) / A
        p[1, _BIAS:_BIAS + 64] = np.asarray(inputs["Wcb"], f32)
        p[0, _CNT:_CNT + 84] = cnt
        p[0, _CNT + 84:_CNT + 168] = cnt
        p[1, _CNT:_CNT + 168] = 1.0
        p[0, _ONES:_ONES + 84] = 1.0
        for j in range(4):
            for m in range(NCHUNK):
                p[_chunk_base(m) + j, _UB + 128 * j + m] = 1.0
        p[0:64, _DECA:_DECA + V] = decw[:, 0:H].T
        p[0:64, _DECB:_DECB + V] = decw[:, H:2 * H].T / 2  # outG/outB hold 2h
        p[64, _DECB:_DECB + V] = np.asarray(inputs["decb"], f32)
        return p.astype(bf)

    pc0 = packc_for(0)
    pcg = packc_for(1)
    pm0 = pmb.copy()
    pm0[0:64, 4] = 1.0
    pm0[0:64, 5] = 0.0
    in_maps = []
    for core in range(NCORES):
        in_maps.append({"packa": pa,
                        "packm": pm0 if core == 0 else pmb,
                        "packc": pc0 if core == 0 else pcg})
    return in_maps


def kernel(**inputs):
    inputs = {k: np.asarray(v) for k, v in inputs.items()}
    Ls = np.asarray(inputs["lengths"]).astype(np.int64)
    steps = int(Ls.max())
    if steps != S or inputs["inputs"].shape != (B, S):
        return _host_reference_fallback(inputs)

    from concourse.bass_utils import run_bass_kernel_spmd

    in_maps = _pack_inputs(inputs)
    nc = _get_nc()
    res = run_bass_kernel_spmd(nc, in_maps, core_ids=list(range(NCORES)))
    outs = np.concatenate(
        [r["out"].astype(np.float32).reshape(BPC, S, V) for r in res.results],
        axis=0)  # [64,512,130]

    bi, ti = np.nonzero(np.arange(steps)[None, :] < (Ls[:, None] - 1))
    return np.ascontiguousarray(outs[bi, ti].reshape(-1, V))
